# revision 45
# baseline (speedup 1.0000x reference)
"""v7 multi-head attention kernel for TRN2, 8-core SPMD.

Problem: qkv (4, 1536, 2048) fp32, 8 heads, ch=64 -> out (4, 512, 2048).
Sharding: 32 head-instances / 8 cores = 4 heads/core as 2 pairs.

Design (PE is the floor: 512 bf16 matmuls x 512 cols = 109.2us/core;
everything else must hide under it; TimelineSim 120.4us total):

- bf16 inputs, host-converted: half the DMA, no f32r restrictions.
- mm1 paired via PE row tiling: a pair's q (and k) live in the partition
  halves of shared [128, *] tiles; two K=64 matmuls per s-tile write
  SEPARATE one-bank PSUM tiles pwA/pwB so the two heads' exp chains are
  fully decoupled.
- exp split BY HEAD each iteration: ACT does head A's 512 cols with
  exact table exp (612ns), DVE does head B's with a 1-instruction
  Schraudolph fast-exp (i16 = trunc(x*A+B) bitcast bf16, 658ns). Both
  run under PE's 853ns/iteration so the exp stage never rate-limits.
  On two slots per chunk (ACT_B_SLOTS) ACT takes head B too, exact.
  Head B is 14/16 Schraudolph: measured rel err 1.166e-2 on the harness
  data vs the 2e-2 gate (all-Schraudolph bound is 1.40e-2).
- PSUM: pwA/pwB (2 bufs x 1 bank each) + po (2 tags x 2 bufs x 1 bank)
  = 8 banks. po double-buffering takes the drain off the critical path.
- mm2 K=128, M=128 with 64 ones-columns (Z lands replicated in po rows
  64:127). Normalization per (head, chunk): ACT copies po rows 64:128
  (zc, rebased to partition 0) off PSUM, DVE reciprocal, DVE multiply
  reading the numerator straight from PSUM, chunked output DMA. These
  ops drain ONE PER ITERATION SLOT starting 2 slots into the next chunk
  (drain_delay) so no engine stream bursts and stalls PE via the pw
  WARs; consumers sit >=1 slot after producers so the in-order SEQs
  never head-block. The final chunk's tails are emitted immediately
  with per-half tiles, both output DMAs on the SP trigger queue.
- v arrives HOST-TRANSPOSED with the ones-columns baked in ("w" input):
  a plain contiguous DMA replaces dma_start_transpose, whose descriptor
  generation costs ~3us per call on the SP sequencer.
- Startup: loads ride one ordered SP queue (HWDGE desc-gen serializes
  globally at ~0.65us/transfer, so queue order IS priority): k/q first
  512 cols (q1 on the ACT trigger queue to overlap), v first s-halves,
  k middle, v rest, k rest, q rest. Five dummy matmuls over a zeroed
  tile burn the initial DMA wait so the p-state ramp reaches 2.4GHz
  before real data lands; all 512 real matmuls run at full speed.
"""

import numpy as np
from contextlib import ExitStack

B = 4
NUM_HEADS = 8
C = 64
T = 2048
N_CORES = 8
HPC = (B * NUM_HEADS) // N_CORES  # 4
R = HPC * C

_SCALE = C ** -0.5  # 0.125
A16 = _SCALE * 1.4426950408889634 * 128.0
B16 = 127.0 * 128.0 - 5.6  # minimax-centered Schraudolph shift

_NC_CACHE = {}

# s-slots (within each chunk) where ACT computes head B's exp instead of
# DVE, freeing those DVE slots for normalization inserts.
ACT_B_SLOTS = (6, 10)


def build_nc(t=T, hpc=HPC, reps=1, la=2, drain_delay=2, act_b=ACT_B_SLOTS, etb_bufs=4, n_warm=5):
    import concourse.mybir as mybir
    import concourse.tile as tile
    from concourse import bacc

    f32 = mybir.dt.float32
    bf16 = mybir.dt.bfloat16
    i16 = mybir.dt.int16
    Exp = mybir.ActivationFunctionType.Exp
    Copy = mybir.ActivationFunctionType.Copy
    Alu = mybir.AluOpType

    st = t // 128
    th = min(512, t)
    n_ch = t // th
    pairs = hpc // 2

    nc = bacc.Bacc("TRN2", debug=False, num_devices=N_CORES)
    q_d = nc.dram_tensor("q", (hpc * C, t), bf16, kind="ExternalInput")
    k_d = nc.dram_tensor("k", (hpc * C, t), bf16, kind="ExternalInput")
    w_d = nc.dram_tensor("w", (hpc * 128, (t // 128) * 128), bf16,
                         kind="ExternalInput")
    o_d = nc.dram_tensor("o", (hpc * C, t), f32, kind="ExternalOutput")

    with tile.TileContext(nc) as tc, ExitStack() as ctx:
        qk_pool = ctx.enter_context(tc.tile_pool(name="qk", bufs=pairs))
        vt_pool = ctx.enter_context(tc.tile_pool(name="vt", bufs=1))
        qp, kp, vt = {}, {}, {}

        def _view(parts, sl):
            # column-range view over the per-range tiles/slices of a pair
            lo = sl.start
            for base, tile_ in parts:
                w = tile_.shape[-1]
                if lo < base + w:
                    return tile_[:, lo - base : sl.stop - base]
            raise AssertionError

        def kview(p, ssl):
            return _view(kp[p], ssl)

        def qview(p, csl):
            return _view(qp[p], csl)

        def emit_loads(p):
            hA, hB = 2 * p, 2 * p + 1
            rsl = slice(hA * 64, hA * 64 + 128)  # pair rows are contiguous
            for h in (hA, hB):
                vt[h] = vt_pool.tile([128, st, 128], bf16, tag=f"vt{h}",
                                     name=f"vt{h}")
            if p == 0 and t > th:
                # Separate tiles per load range: a single [128, t] tile
                # with several DMA writers makes the dep tracker merge the
                # write regions, so a read of cols [512:640] ends up
                # waiting on the LAST k transfer. One tile per transfer
                # keeps every dependency exact. Queue order is priority
                # (HWDGE serializes globally); q1 rides the ACT queue to
                # overlap with SP's k1 trigger.
                sh = st // 2
                ka = qk_pool.tile([128, th], bf16, tag="ka", name=f"ka{p}")
                kb = qk_pool.tile([128, th], bf16, tag="kb", name=f"kb{p}")
                kc = qk_pool.tile([128, t - 2 * th], bf16, tag="kc",
                                  name=f"kc{p}")
                qa = qk_pool.tile([128, th], bf16, tag="qa", name=f"qa{p}")
                qb = qk_pool.tile([128, t - th], bf16, tag="qb",
                                  name=f"qb{p}")
                kp[p] = [(0, ka), (th, kb), (2 * th, kc)]
                qp[p] = [(0, qa), (th, qb)]
                nc.sync.dma_start(out=ka, in_=k_d[rsl, 0:th])
                nc.scalar.dma_start(out=qa, in_=q_d[rsl, 0:th])
                # v arrives host-transposed with the ones-columns baked in
                # (plain contiguous DMA: the on-chip transpose costs ~3us
                # of descriptor generation per call). First s-halves land
                # ahead of kb: mm2(0) needs them before mm1 needs kb.
                wcols = sh * 128
                for h in (hA, hB):
                    nc.sync.dma_start(out=vt[h][:, 0:sh, :],
                                      in_=w_d[h * 128 : h * 128 + 128,
                                              0:wcols])
                nc.sync.dma_start(out=kb, in_=k_d[rsl, th : 2 * th])
                for h in (hA, hB):
                    nc.sync.dma_start(out=vt[h][:, sh:st, :],
                                      in_=w_d[h * 128 : h * 128 + 128,
                                              wcols : st * 128])
                nc.sync.dma_start(out=kc, in_=k_d[rsl, 2 * th : t])
                nc.sync.dma_start(out=qb, in_=q_d[rsl, th:t])
            else:
                kw = qk_pool.tile([128, t], bf16, tag="k", name=f"kp{p}")
                qw = qk_pool.tile([128, t], bf16, tag="q", name=f"qp{p}")
                kp[p] = [(0, kw)]
                qp[p] = [(0, qw)]
                nc.sync.dma_start(out=kw, in_=k_d[rsl, :])
                nc.sync.dma_start(out=qw, in_=q_d[rsl, :])
                for h in (hA, hB):
                    nc.sync.dma_start(out=vt[h],
                                      in_=w_d[h * 128 : h * 128 + 128, :])

        for p in range(pairs):
            emit_loads(p)

        pwa_pool = ctx.enter_context(tc.tile_pool(name="pwa", bufs=la, space="PSUM"))
        pwb_pool = ctx.enter_context(tc.tile_pool(name="pwb", bufs=la, space="PSUM"))
        po_pool = ctx.enter_context(tc.tile_pool(name="po", bufs=2, space="PSUM"))
        eta_pool = ctx.enter_context(tc.tile_pool(name="eta", bufs=etb_bufs))
        etb_pool = ctx.enter_context(tc.tile_pool(name="etb", bufs=etb_bufs))
        nrm_pool = ctx.enter_context(tc.tile_pool(name="nrm", bufs=4))
        osb_pool = ctx.enter_context(tc.tile_pool(name="osb", bufs=4))

        flat = [
            (rep, p, ci, s)
            for rep in range(reps)
            for p in range(pairs)
            for ci in range(n_ch)
            for s in range(st)
        ]
        n = len(flat)

        pw_tiles, et_tiles = {}, {}

        def emit_mm1(i):
            rep, p, ci, s = flat[i]
            ssl = slice(s * 128, (s + 1) * 128)
            csl = slice(ci * th, ci * th + th)
            kv = kview(p, ssl)
            qv = qview(p, csl)
            pwa = pwa_pool.tile([128, th], f32, name="pwa")
            pwb = pwb_pool.tile([128, th], f32, name="pwb")
            nc.tensor.matmul(pwa, kv[0:64, :], qv[0:64, :],
                             start=True, stop=True)
            nc.tensor.matmul(pwb, kv[64:128, :], qv[64:128, :],
                             start=True, stop=True)
            pw_tiles[i] = (pwa, pwb)

        def emit_exp(i):
            rep, p, ci, s = flat[i]
            pwa, pwb = pw_tiles.pop(i)
            eta = eta_pool.tile([128, th], bf16, name="eta")
            etb = etb_pool.tile([128, th], bf16, name="etb")
            et_tiles[i] = (eta, etb)
            nc.scalar.activation(out=eta, in_=pwa, func=Exp, scale=_SCALE)
            if s in act_b:
                nc.scalar.activation(out=etb, in_=pwb, func=Exp, scale=_SCALE)
            else:
                nc.vector.tensor_scalar(
                    out=etb.bitcast(i16),
                    in0=pwb,
                    scalar1=A16, scalar2=B16,
                    op0=Alu.mult, op1=Alu.add,
                )

        po_cur = [None, None]
        tail_slots = []  # list of op-lists; one list drained per iteration

        def emit_mm2(i, half):
            rep, p, ci, s = flat[i]
            h = 2 * p + half
            et = et_tiles[i][half]
            if s == 0:
                po_cur[half] = po_pool.tile([128, th], f32, tag=f"po{half}",
                                            name=f"po{half}")
            nc.tensor.matmul(
                po_cur[half], vt[h][:, s, :], et,
                start=(s == 0), stop=(s == st - 1),
            )
            if s == st - 1:
                if i == n - 1:
                    emit_tail_fast(h, ci, po_cur[half])
                elif half == 1:
                    queue_tail(2 * p, 2 * p + 1, ci, po_cur[0], po_cur[1],
                               compress=(i == n - 17))

        def queue_tail(hL, hH, ci, poL, poH, compress=False):
            t0 = ci * th
            state = {}

            def op_zc(po, key):
                def f():
                    zc = nrm_pool.tile([64, th], f32, tag="zc", name="zc")
                    nc.scalar.activation(out=zc, in_=po[64:128, :], func=Copy)
                    state[("zc", key)] = zc
                return f

            def op_recip(key):
                def f():
                    rz = nrm_pool.tile([64, th], f32, tag="rz", name="rz")
                    nc.vector.reciprocal_approx_fast(
                        out=rz, in_=state[("zc", key)])
                    state[("rz", key)] = rz
                return f

            def op_mul(po, key):
                def f():
                    osb = osb_pool.tile([64, th], f32, name="osb")
                    # numerator read straight from PSUM; frees the po bank
                    nc.vector.tensor_mul(osb, po[0:64, :], state[("rz", key)])
                    state[("osb", key)] = osb
                return f

            def op_dma(h, key):
                def f():
                    nc.sync.dma_start(
                        out=o_d[h * 64 : h * 64 + 64, t0 : t0 + th],
                        in_=state[("osb", key)],
                    )
                return f

            # One op per iteration slot, consumer >=1 slot after producer
            # (in-order SEQs head-block on fresh deps otherwise).
            tail_slots.extend([[] for _ in range(drain_delay)])
            tail_slots.extend([[op] for op in [
                op_zc(poL, "L"), op_recip("L"), op_mul(poL, "L"),
                op_dma(hL, "L"),
                op_zc(poH, "H"), op_recip("H"), op_mul(poH, "H"),
                op_dma(hH, "H"),
            ]])

        def drain_tail_ops(k=1):
            for _ in range(min(k, len(tail_slots))):
                for op in tail_slots.pop(0):
                    op()

        def emit_tail_fast(h, ci, po):
            # Final-chunk tail: everything is latency-critical, so pipeline
            # in column halves and fire each output DMA as soon as its half
            # of the multiply lands. Separate tiles per half: shared tiles
            # make the dep tracker serialize the two halves' chains.
            t0 = ci * th
            # Full-width ops (the DMA engine costs ~0.7us per transfer
            # regardless of size, so one DMA per head wins), separate
            # tiles per call, po readers emitted zc-before-mul (reader
            # serialization is then harmless: mul needs recip(zc) anyway).
            zc = nrm_pool.tile([64, th], f32, tag="zcf", name="zcf")
            nc.scalar.activation(out=zc, in_=po[64:128, :], func=Copy)
            rz = nrm_pool.tile([64, th], f32, tag="rzf", name="rzf")
            nc.vector.reciprocal_approx_fast(out=rz, in_=zc)
            osb = osb_pool.tile([64, th], f32, tag="of", name="osbf")
            nc.vector.tensor_mul(osb, po[0:64, :], rz)
            eng = nc.sync
            eng.dma_start(
                out=o_d[h * 64 : h * 64 + 64, t0 : t0 + th],
                in_=osb,
            )

        if t > 512:
            # PE p-state warmup: the ramp model runs the first real matmuls
            # at 0.65-1.2GHz; burn the DMA-wait time on dummy matmuls over a
            # zeroed tile so the ramp reaches full speed before data lands.
            wz = qk_pool.tile([128, 512], bf16, tag="wz", name="wz")
            nc.gpsimd.memset(wz, 0.0)
            for wi in range(n_warm):
                pww = pwa_pool.tile([128, th], f32, name="pwa")
                nc.tensor.matmul(pww, wz[0:64, 0:128], wz[0:64, :],
                                 start=True, stop=True)
        for j in range(min(la, n)):
            emit_mm1(j)
        for i in range(n):
            # exp(i) BEFORE mm1(i+la): iteration i+la reuses pw slot i%la -
            # the exp read must be emitted first so the WAR dep is tracked.
            emit_exp(i)
            if i + la < n:
                emit_mm1(i + la)
            drain_tail_ops(1)
            emit_mm2(i, 0)
            emit_mm2(i, 1)
            del et_tiles[i]
        drain_tail_ops(len(tail_slots))

    nc.compile()
    return nc


def get_nc(**kw):
    key = tuple(sorted(kw.items()))
    if key not in _NC_CACHE:
        _NC_CACHE[key] = build_nc(**kw)
    return _NC_CACHE[key]


def _bf16(x):
    import ml_dtypes

    return np.asarray(x, np.float32).astype(ml_dtypes.bfloat16)


def make_w(v):
    # v: (HPC*64, T) fp32 -> w: (HPC*128, (T//128)*128) bf16 where
    # w[h*128+k, s*128*... ] holds [v^T s-tile | ones] per s, matching the
    # mm2 stationary layout vt[h][k, s, m].
    st = T // 128
    vr = v.reshape(HPC, C, st, 128)              # (h, c, s, k)
    w = np.ones((HPC, 128, st, 128), np.float32)
    w[:, :, :, 0:64] = np.transpose(vr, (0, 3, 2, 1))  # (h, k, s, c)
    return _bf16(w.reshape(HPC * 128, st * 128))


def make_in_maps(qkv):
    qkv = np.ascontiguousarray(np.asarray(qkv, np.float32))
    in_maps = []
    for m in range(N_CORES):
        b = m // 2
        r0 = HPC * C * (m % 2)
        in_maps.append(
            {
                "q": _bf16(qkv[b, r0 : r0 + R, :]),
                "k": _bf16(qkv[b, 512 + r0 : 512 + r0 + R, :]),
                "w": make_w(qkv[b, 1024 + r0 : 1024 + r0 + R, :]),
            }
        )
    return in_maps


def assemble_out(results):
    out = np.empty((B, NUM_HEADS * C, T), dtype=np.float32)
    for m in range(N_CORES):
        b = m // 2
        r0 = HPC * C * (m % 2)
        out[b, r0 : r0 + R, :] = results[m]["o"]
    return out


def kernel(qkv):
    from concourse.bass_utils import run_bass_kernel_spmd

    nc = get_nc()
    in_maps = make_in_maps(qkv)
    res = run_bass_kernel_spmd(nc, in_maps, core_ids=list(range(N_CORES)))
    return assemble_out(res.results)
